# revision 69
# baseline (speedup 1.0000x reference)
"""Causal self-attention kernel for 8 Trainium2 NeuronCores.

Problem: B=4, T=2048, C=1024, NH=16, HD=64 (fp32 in/out).

Sharding: core c = (batch b = c//2, head-group g = c%2 of 8 heads).
Per core everything is computed transposed (feature-major) so no on-device
transposes are needed.  All matmul operands are bf16 (inputs are cast on the
host; PSUM accumulation stays fp32), which halves HBM traffic and SBUF
footprint at the same PE rate as fp32r.

v2 changes vs v1 (548us -> target ~300us):
  - softmax normalization is fully on-chip: DVE reciprocal of the Z row
    (PSUM, partition 64) into SBUF, then one SBUF->SBUF partition-broadcast
    DMA issued from the GPSIMD queue (v1 bounced Z through DRAM on the Sync
    queue, where it head-of-line blocked behind the ReduceScatter for up to
    25us per chunk boundary).
  - causal trapezoid: on diagonal key-blocks the scores/exp/AV free range is
    clipped to the valid queries [128r:512]; masking shrinks to one 128x128
    lower-triangular multiply per (p, diag-block) over both heads.
  - ReduceScatter is sliced into 4 per-(oc,oc+4) collectives per chunk,
    issued mid-chunk right after the corresponding projection columns land,
    and writes straight into the output DRAM tensor (no cc_out, no out DMA).
  - startup DMAs are ordered so the first matmul can start after ~2MB
    (xt chunk 0 + W_v) instead of after all weights.
  - the chunk-0 qkv prologue ping-pongs between two PSUM banks (ps_acc and a
    borrowed scores bank) instead of serializing on one.
"""

import numpy as np
import ml_dtypes
from contextlib import ExitStack

import concourse.bass as bass
import concourse.tile as tile
import concourse.mybir as mybir
from concourse import bacc
from concourse.bass_utils import run_bass_kernel_spmd

B, C, NH, HD = 4, 1024, 16, 64
NCORES = 8
NP = 4              # head pairs per core (8 heads)
QC = 512            # query-chunk (free dim of most matmuls)
KB = 128            # key block (partition dim of score blocks)
CCH = C // 128      # 8 contraction chunks
FP32 = mybir.dt.float32
BF16 = mybir.dt.bfloat16
EXP = mybir.ActivationFunctionType.Exp
GROUPS = [[0, 1], [2, 3], [4, 5], [6, 7]]
BF = ml_dtypes.bfloat16


def build_program(T=2048, mode="full"):
    nqc = T // QC
    nc = bacc.Bacc("TRN2", target_bir_lowering=False, debug=False,
                   num_devices=NCORES)

    xt_d = nc.dram_tensor("xt", [C, T], BF16, kind="ExternalInput").ap()
    wqk_d = nc.dram_tensor("wqk", [C, C], BF16, kind="ExternalInput").ap()
    bqk_d = nc.dram_tensor("bqk", [C], FP32, kind="ExternalInput").ap()
    wv_d = nc.dram_tensor("wv", [C, 512], BF16, kind="ExternalInput").ap()
    # bv arrives host-pre-broadcast over partitions: [128, 8, 64]
    bv_d = nc.dram_tensor("bv", [128, 8, HD], FP32, kind="ExternalInput").ap()
    # wp: all 1024 W_proj rows, this core's 512 output columns
    wp_d = nc.dram_tensor("wp", [C, 512], BF16, kind="ExternalInput").ap()
    bp_d = nc.dram_tensor("bp", [512], FP32, kind="ExternalInput").ap()
    # lower-triangular [128, 2, 128] mask (duplicated over the e axis)
    mask_d = nc.dram_tensor("mask", [128, 2, 128], BF16,
                            kind="ExternalInput").ap()
    # chunk-major bf16 output: out_t[qc] = this core's 512 outT rows for
    # queries [qc*QC, (qc+1)*QC); host casts back to fp32
    out_d = nc.dram_tensor("out_t", [nqc, 512, QC], BF16,
                           kind="ExternalOutput").ap()
    # attention outputs are exchanged between pair cores by AllGather of y
    # (copy-only — much cheaper on the CC cores than ReduceScatter of proj
    # partials); each core then runs the full-1024-contraction projection
    # for its own 512 output features.  ag block 0 = heads 0-7 (the g=0
    # core's y), block 1 = heads 8-15 — identical layout on both cores.
    # the exchange is split: AG-A carries head pairs 0-1 (triggered ~50%
    # through the chunk, so it always lands before the projection needs it),
    # AG-B carries pairs 2-3 at chunk end
    yx_d = [nc.dram_tensor(f"yx{q}", [512, QC], BF16).ap() for q in range(nqc)]
    aga_d = [nc.dram_tensor(f"aga{q}", [512, QC], BF16).ap()
             for q in range(nqc)]
    agb_d = [nc.dram_tensor(f"agb{q}", [512, QC], BF16).ap()
             for q in range(nqc)]
    zdr = nc.dram_tensor("zdr", [8, QC], FP32).ap()

    with tile.TileContext(nc) as tc, ExitStack() as ctx:
        resid = ctx.enter_context(tc.tile_pool(name="resid", bufs=1))
        xtp = ctx.enter_context(tc.tile_pool(name="xtp", bufs=2))
        qp = ctx.enter_context(tc.tile_pool(name="qp", bufs=2))
        yp = ctx.enter_context(tc.tile_pool(name="yp", bufs=2))
        ep = ctx.enter_context(tc.tile_pool(name="ep", bufs=2))
        rp = ctx.enter_context(tc.tile_pool(name="rp", bufs=4))
        op = ctx.enter_context(tc.tile_pool(name="op", bufs=2))
        agp = ctx.enter_context(tc.tile_pool(name="agp", bufs=2))
        ps_acc = ctx.enter_context(tc.tile_pool(name="ps_acc", bufs=1, space="PSUM"))
        ps_s = ctx.enter_context(tc.tile_pool(name="ps_s", bufs=2, space="PSUM"))
        ps_y = ctx.enter_context(tc.tile_pool(name="ps_y", bufs=3, space="PSUM"))

        # ---- startup DMAs: per-contraction-chunk tiles (fine-grained deps
        # so the first matmul starts after one slice pair, not the full set)
        # spread over three issue queues (sync: xt/bias, gpsimd: wv,
        # scalar: wqk) so the rings run in parallel
        bv_bc = resid.tile([128, 8, HD], FP32, name="bv_bc")
        xt0 = [xtp.tile([128, QC], BF16, name=f"xt{cch}", tag=f"xt{cch}")
               for cch in range(CCH)]
        wv_t = [resid.tile([128, 512], BF16, name=f"wv{cch}")
                for cch in range(CCH)]
        wqk_t = [resid.tile([128, 8, 128], BF16, name=f"wqk{cch}")
                 for cch in range(CCH)]
        for cch in range(CCH):
            nc.sync.dma_start(out=xt0[cch],
                              in_=xt_d[cch * 128:(cch + 1) * 128, 0:QC])
            nc.sync.dma_start(out=wv_t[cch],
                              in_=wv_d[cch * 128:(cch + 1) * 128, :])
            nc.scalar.dma_start(
                out=wqk_t[cch],
                in_=wqk_d[cch * 128:(cch + 1) * 128, :].rearrange(
                    "p (f n) -> p f n", f=8))
        nc.sync.dma_start(out=bv_bc, in_=bv_d)
        bqk_sb = resid.tile([128, 8], FP32, name="bqk_sb")
        nc.sync.dma_start(out=bqk_sb, in_=bqk_d.rearrange("(f p) -> p f", p=128))

        mask = resid.tile([128, 2, 128], BF16, name="mask")
        nc.sync.dma_start(out=mask, in_=mask_d)
        wp_sb = resid.tile([128, CCH, 4, 128], BF16, name="wp_sb")
        for cch in range(CCH):
            nc.sync.dma_start(
                out=wp_sb[:, cch],
                in_=wp_d[cch * 128:(cch + 1) * 128, :].rearrange(
                    "p (f n) -> p f n", f=4))
        bp_sb = resid.tile([128, 4], FP32, name="bp_sb")
        nc.sync.dma_start(out=bp_sb, in_=bp_d.rearrange("(f p) -> p f", p=128))

        ksb = [resid.tile([128, T], BF16, name=f"ksb{p}") for p in range(NP)]
        vsb = [resid.tile([128, 8, HD + 1], BF16, name=f"vsb{tb}")
               for tb in range(T // 128)]
        for tb in range(T // 128):
            nc.gpsimd.memset(vsb[tb][:, :, HD:HD + 1], 1.0)

        # ---------- emission helpers ----------
        def load_xt(qc):
            xt_sb = [xtp.tile([128, QC], BF16, name=f"xt{cch}",
                              tag=f"xt{cch}") for cch in range(CCH)]
            for cch in range(CCH):
                nc.sync.dma_start(
                    out=xt_sb[cch],
                    in_=xt_d[cch * 128:(cch + 1) * 128, qc * QC:(qc + 1) * QC])
            return xt_sb

        def acc_tile(alt):
            # ping-pong accumulator: ps_acc, or a borrowed half of a scores
            # bank (only safe where the scores pipeline isn't running hot)
            if alt % 2 == 0:
                return ps_acc.tile([128, QC], FP32, name="pacc", tag="pv")
            return ps_s.tile([128, 2 * QC], FP32, name="sps", tag="sps")[:, 0:QC]

        def emit_v(xt_sb, qc, j, alt=0):
            tb = qc * (QC // 128) + j
            pv = acc_tile(alt)
            for cch in range(CCH):
                nc.tensor.matmul(
                    out=pv, lhsT=xt_sb[cch][:, j * 128:(j + 1) * 128],
                    rhs=wv_t[cch], start=(cch == 0), stop=(cch == CCH - 1))
            nc.vector.tensor_add(
                vsb[tb][:, :, 0:HD],
                pv.rearrange("p (l d) -> p l d", l=8), bv_bc)

        def emit_qk(xt_sb, q_sb, qc, f, alt=0):
            pqk = acc_tile(alt)
            for cch in range(CCH):
                nc.tensor.matmul(
                    out=pqk, lhsT=wqk_t[cch][:, f], rhs=xt_sb[cch],
                    start=(cch == 0), stop=(cch == CCH - 1))
            p, isk = f // 2, f % 2
            dst = (ksb[p][:, qc * QC:(qc + 1) * QC] if isk else q_sb[p])
            nc.vector.tensor_scalar_add(dst, pqk, bqk_sb[:, f:f + 1])

        def emit_ag(qc, half):
            src = yx_d[qc][0:256, :] if half == 0 else yx_d[qc][256:512, :]
            dst = aga_d[qc] if half == 0 else agb_d[qc]
            nc.gpsimd.collective_compute(
                "AllGather", mybir.AluOpType.bypass, replica_groups=GROUPS,
                ins=[src], outs=[dst])

        # contraction slot order is rank-major = natural head order:
        # slots 0-1 rank0 pairs 0-1 (AG-A lo), 2-3 rank0 pairs 2-3 (AG-B lo),
        # 4-5 rank1 pairs 0-1 (AG-A hi), 6-7 rank1 pairs 2-3 (AG-B hi)
        def ag_slice(qc, cch):
            src = aga_d[qc] if (cch % 4) < 2 else agb_d[qc]
            off = (cch % 2) + 2 * (cch // 4)
            return src[off * 128:(off + 1) * 128, :]

        def load_ag(qc, cchs=range(CCH), scalar_ok=False):
            # separate tiles so each projection matmul only waits for its
            # own contraction slice.  Queue: gpsimd (idle except AG
            # triggers); in the epilogue the scalar queue is idle too.
            tiles = {}
            for cch in cchs:
                t = agp.tile([128, QC], BF16, name=f"ag{cch}", tag=f"ag{cch}")
                eng = nc.scalar if scalar_ok else nc.gpsimd
                eng.dma_start(out=t, in_=ag_slice(qc, cch))
                tiles[cch] = t
            return tiles

        def proj_pass(pp, ag_sb, oc, cchs, start, stop):
            for k, cch in enumerate(cchs):
                nc.tensor.matmul(out=pp, lhsT=wp_sb[:, cch, oc],
                                 rhs=ag_sb[cch],
                                 start=(start and k == 0),
                                 stop=(stop and k == len(cchs) - 1))

        def proj_evac(pp, qc, oc):
            po = op.tile([128, QC], BF16, name="po")
            nc.vector.tensor_scalar_add(po, pp, bp_sb[:, oc:oc + 1])
            nc.sync.dma_start(out=out_d[qc, oc * 128:(oc + 1) * 128, :],
                              in_=po)

        def emit_proj(ag_sb, qc, oc, alt=0):
            pp = acc_tile(alt)
            proj_pass(pp, ag_sb, oc, range(CCH), True, True)
            proj_evac(pp, qc, oc)

        def new_q():
            return [qp.tile([128, QC], BF16, name=f"qsb{p}", tag=f"qsb{p}")
                    for p in range(NP)]

        # ---------- prologue: chunk 0 qkv projection (ping-pong banks) ----
        xt_cur = xt0
        q_cur = new_q()
        alt = 0
        for j in range(QC // 128):
            emit_v(xt_cur, 0, j, alt); alt += 1
        for f in range(8):
            emit_qk(xt_cur, q_cur, 0, f, alt); alt += 1

        y_prev = None
        gstep = [0]
        dq = []     # deferred vector ops: (ready_gstep, fn) — issued a few
                    # steps after their z-broadcast DMA so they never sit
                    # waiting at the head of the vector queue

        def pop_dq():
            while dq and dq[0][0] <= gstep[0]:
                dq.pop(0)[1]()

        def flush_dq():
            while dq:
                dq.pop(0)[1]()

        for qc in range(nqc):
            # background PE units interleaved into this chunk's attention:
            # the previous chunk's AllGather-result load first (its waits sit
            # on the idle gpsimd queue), then next chunk's qkv, then the
            # previous chunk's projection (by which time the AG has landed)
            bg = []
            if qc > 0:
                ag_sb = load_ag(qc - 1)
            if qc + 1 < nqc:
                xt_nxt = load_xt(qc + 1)
                q_nxt = new_q()
                for j in range(QC // 128):
                    bg.append((emit_v, (xt_nxt, qc + 1, j)))
                for f in range(8):
                    bg.append((emit_qk, (xt_nxt, q_nxt, qc + 1, f)))
            else:
                xt_nxt, q_nxt = None, None
            if qc > 0:
                for oc in range(4):
                    bg.append((emit_proj, (ag_sb, qc - 1, oc)))
            bg_total = len(bg)

            njb = 4 * (qc + 1)
            steps = NP * njb
            # drain bg by ~7/8 of the chunk so end-of-chunk vector work
            # (normalize + AllGather feed) isn't stuck behind bg evacuations
            pace = max(1, (7 * steps) // 8)
            y_cur = [yp.tile([128, QC], BF16, name=f"y{p}", tag=f"y{p}")
                     for p in range(NP)]
            step = 0
            for p in range(NP):
                yps = [ps_y.tile([HD + 1, QC], FP32, name=f"yps{e}", tag="yps")
                       for e in (0, 1)]
                for jb in range(njb):
                    r = jb - 4 * qc
                    lo = max(0, 128 * r)      # first valid query column
                    sps = ps_s.tile([128, 2 * QC], FP32, name="sps", tag="sps")
                    for e in (0, 1):
                        nc.tensor.matmul(
                            out=sps[:, e * QC + lo:(e + 1) * QC],
                            lhsT=ksb[p][e * HD:(e + 1) * HD,
                                        jb * KB:(jb + 1) * KB],
                            rhs=q_cur[p][e * HD:(e + 1) * HD, lo:QC],
                            start=True, stop=True)
                    esb = ep.tile([128, 2, QC], BF16, name="esb")
                    nc.scalar.activation(
                        out=esb[:, :, lo:QC],
                        in_=sps.rearrange("p (e n) -> p e n", e=2)[:, :, lo:QC],
                        func=EXP, scale=0.125)
                    if r >= 0:
                        nc.vector.tensor_mul(
                            esb[:, :, lo:lo + 128], esb[:, :, lo:lo + 128],
                            mask)
                    for e in (0, 1):
                        nc.tensor.matmul(
                            out=yps[e][:, lo:QC], lhsT=vsb[jb][:, 2 * p + e, :],
                            rhs=esb[:, e, lo:QC],
                            start=(jb == 0), stop=(jb == njb - 1))
                    # keep the in-order PE stream dense: spread background
                    # units evenly across the attention steps
                    step += 1
                    gstep[0] += 1
                    pop_dq()
                    while bg and len(bg) > bg_total * max(0, pace - step) // pace:
                        fn, args = bg.pop(0)
                        fn(*args)
                # normalize: evacuate yps to SBUF right away (frees the PSUM
                # slots before the next p's matmuls need them), kick off the
                # Z DRAM-bounce broadcast, and defer reciprocal+multiply
                for e in (0, 1):
                    ysb = rp.tile([HD + 1, QC], FP32, name="ysb", tag="ysb")
                    nc.vector.tensor_copy(ysb, yps[e])
                    slot = zdr[2 * p + e]
                    nc.sync.dma_start(out=slot, in_=ysb[HD:HD + 1, :])
                    rzb = rp.tile([HD, QC], FP32, name="rzb", tag="rzb")
                    nc.sync.dma_start(out=rzb,
                                      in_=slot.partition_broadcast(HD))

                    def mk(ysb=ysb, rzb=rzb, p=p, e=e, y_cur=y_cur, qc=qc):
                        def go():
                            nc.vector.reciprocal_approx_fast(rzb, rzb)
                            nc.vector.tensor_mul(
                                y_cur[p][e * HD:(e + 1) * HD, :],
                                ysb[0:HD, :], rzb)
                            nc.sync.dma_start(
                                out=yx_d[qc][p * 128 + e * HD:
                                             p * 128 + (e + 1) * HD, :],
                                in_=y_cur[p][e * HD:(e + 1) * HD, :])
                            if p == 1 and e == 1:
                                emit_ag(qc, 0)
                        return go
                    dq.append((gstep[0] + 3, mk()))
            for fn, args in bg:
                fn(*args)
            flush_dq()
            emit_ag(qc, 1)
            y_prev = y_cur
            xt_cur, q_cur = xt_nxt, q_nxt

        # epilogue, two passes: the AG-A slices (own/peer pairs 0-2) project
        # while pair 3's normalize chain and its small AG-B run; pass 2 adds
        # the two pair-3 slices.  4 simultaneously-open PSUM groups (the
        # attention banks are free by now).
        A_CCHS, B_CCHS = [0, 1, 4, 5], [2, 3, 6, 7]
        lq = nqc - 1
        ag_sb = load_ag(lq, A_CCHS, scalar_ok=True)
        spsA = ps_s.tile([128, 2 * QC], FP32, name="sps", tag="sps")
        spsB = ps_s.tile([128, 2 * QC], FP32, name="sps", tag="sps")
        pps = [ps_acc.tile([128, QC], FP32, name="pacc", tag="pv"),
               spsA[:, 0:QC], spsA[:, QC:2 * QC], spsB[:, 0:QC]]
        for oc in range(4):
            proj_pass(pps[oc], ag_sb, oc, A_CCHS, True, False)
        ag_sb.update(load_ag(lq, B_CCHS, scalar_ok=True))
        for oc in range(4):
            proj_pass(pps[oc], ag_sb, oc, B_CCHS, False, True)
            proj_evac(pps[oc], lq, oc)


    nc.compile()
    return nc


def shard_inputs(x, W_attn, b_attn, W_proj, b_proj):
    in_maps = []
    u = np.arange(128)[None, :]
    p_ = np.arange(128)[:, None]
    tri = (p_ <= u).astype(BF)                     # [128, 128] lower-tri
    mask_np = np.ascontiguousarray(
        np.broadcast_to(tri[:, None, :], (128, 2, 128)))
    wp_perm = np.concatenate([np.arange(0, 256), np.arange(512, 768),
                              np.arange(256, 512), np.arange(768, 1024)])
    for c in range(NCORES):
        b, g = c // 2, c % 2
        xt = np.ascontiguousarray(x[b].T.astype(BF))
        # w_qk columns: feat chunk f = 2p+isK holds q (isK=0) or k (isK=1)
        # features of heads (8g+2p, 8g+2p+1)
        qk_idx = []
        for f in range(8):
            p, isk = f // 2, f % 2
            for e in (0, 1):
                h = 8 * g + 2 * p + e
                base = isk * C + h * HD
                qk_idx.append(np.arange(base, base + HD))
        qk_idx = np.concatenate(qk_idx)
        v_idx = np.concatenate(
            [np.arange(2 * C + (8 * g + l) * HD, 2 * C + (8 * g + l) * HD + HD)
             for l in range(8)])
        in_maps.append({
            "mask": mask_np,
            "xt": xt,
            "wqk": np.ascontiguousarray(W_attn[:, qk_idx].astype(BF)),
            "bqk": np.ascontiguousarray(b_attn[qk_idx].astype(np.float32)),
            "wv": np.ascontiguousarray(W_attn[:, v_idx].astype(BF)),
            "bv": np.ascontiguousarray(np.broadcast_to(
                b_attn[v_idx].astype(np.float32).reshape(8, HD),
                (128, 8, HD))),
            # full contraction rows (ag slots are rank-ordered = natural
            # head order on both cores); this core's output columns
            "wp": np.ascontiguousarray(
                W_proj[:, g * 512:(g + 1) * 512].astype(BF)),
            "bp": np.ascontiguousarray(
                b_proj[g * 512:(g + 1) * 512].astype(np.float32)),
        })
    return in_maps


def assemble_output(results, T):
    out = np.empty((B, T, C), np.float32)
    nqc = T // QC
    for b in range(B):
        for g in range(2):
            ot = np.asarray(results[2 * b + g]["out_t"], dtype=np.float32)
            for qc in range(nqc):
                out[b, qc * QC:(qc + 1) * QC, g * 512:(g + 1) * 512] = \
                    ot[qc].T
    return out


_PROG = {}


def _get_program(T, mode="full"):
    key = (T, mode)
    if key not in _PROG:
        _PROG[key] = build_program(T, mode)
    return _PROG[key]


def run_sharded(inputs, trace=False, mode="full"):
    """Returns (output [B,T,C], BassKernelResults)."""
    x = np.asarray(inputs["x"])
    T = x.shape[1]
    nc = _get_program(T, mode)
    in_maps = shard_inputs(x, np.asarray(inputs["W_attn"]),
                           np.asarray(inputs["b_attn"]),
                           np.asarray(inputs["W_proj"]),
                           np.asarray(inputs["b_proj"]))
    res = run_bass_kernel_spmd(nc, in_maps, list(range(NCORES)), trace=trace)
    return assemble_output(res.results, T), res


def kernel(**inputs):
    out, _ = run_sharded(inputs)
    return out


# revision 70
# speedup vs baseline: 1.0330x; 1.0330x over previous
"""Causal self-attention kernel for 8 Trainium2 NeuronCores.

Problem: B=4, T=2048, C=1024, NH=16, HD=64 (fp32 in/out).

Sharding: core c = (batch b = c//2, head-group g = c%2 of 8 heads).
Per core everything is computed transposed (feature-major) so no on-device
transposes are needed.  All matmul operands are bf16 (inputs are cast on the
host; PSUM accumulation stays fp32), which halves HBM traffic and SBUF
footprint at the same PE rate as fp32r.

v2 changes vs v1 (548us -> target ~300us):
  - softmax normalization is fully on-chip: DVE reciprocal of the Z row
    (PSUM, partition 64) into SBUF, then one SBUF->SBUF partition-broadcast
    DMA issued from the GPSIMD queue (v1 bounced Z through DRAM on the Sync
    queue, where it head-of-line blocked behind the ReduceScatter for up to
    25us per chunk boundary).
  - causal trapezoid: on diagonal key-blocks the scores/exp/AV free range is
    clipped to the valid queries [128r:512]; masking shrinks to one 128x128
    lower-triangular multiply per (p, diag-block) over both heads.
  - ReduceScatter is sliced into 4 per-(oc,oc+4) collectives per chunk,
    issued mid-chunk right after the corresponding projection columns land,
    and writes straight into the output DRAM tensor (no cc_out, no out DMA).
  - startup DMAs are ordered so the first matmul can start after ~2MB
    (xt chunk 0 + W_v) instead of after all weights.
  - the chunk-0 qkv prologue ping-pongs between two PSUM banks (ps_acc and a
    borrowed scores bank) instead of serializing on one.
"""

import numpy as np
import ml_dtypes
from contextlib import ExitStack

import concourse.bass as bass
import concourse.tile as tile
import concourse.mybir as mybir
from concourse import bacc
from concourse.bass_utils import run_bass_kernel_spmd

B, C, NH, HD = 4, 1024, 16, 64
NCORES = 8
NP = 4              # head pairs per core (8 heads)
QC = 512            # query-chunk (free dim of most matmuls)
KB = 128            # key block (partition dim of score blocks)
CCH = C // 128      # 8 contraction chunks
FP32 = mybir.dt.float32
BF16 = mybir.dt.bfloat16
EXP = mybir.ActivationFunctionType.Exp
GROUPS = [[0, 1], [2, 3], [4, 5], [6, 7]]
BF = ml_dtypes.bfloat16


def build_program(T=2048, mode="full"):
    nqc = T // QC
    nc = bacc.Bacc("TRN2", target_bir_lowering=False, debug=False,
                   num_devices=NCORES)

    xt_d = nc.dram_tensor("xt", [C, T], BF16, kind="ExternalInput").ap()
    wqk_d = nc.dram_tensor("wqk", [C, C], BF16, kind="ExternalInput").ap()
    bqk_d = nc.dram_tensor("bqk", [C], FP32, kind="ExternalInput").ap()
    wv_d = nc.dram_tensor("wv", [C, 512], BF16, kind="ExternalInput").ap()
    # bv arrives host-pre-broadcast over partitions: [128, 8, 64]
    bv_d = nc.dram_tensor("bv", [128, 8, HD], FP32, kind="ExternalInput").ap()
    # wp: all 1024 W_proj rows, this core's 512 output columns
    wp_d = nc.dram_tensor("wp", [C, 512], BF16, kind="ExternalInput").ap()
    bp_d = nc.dram_tensor("bp", [512], FP32, kind="ExternalInput").ap()
    # lower-triangular [128, 2, 128] mask (duplicated over the e axis)
    mask_d = nc.dram_tensor("mask", [128, 2, 128], BF16,
                            kind="ExternalInput").ap()
    # chunk-major bf16 output: out_t[qc] = this core's 512 outT rows for
    # queries [qc*QC, (qc+1)*QC); host casts back to fp32
    out_d = nc.dram_tensor("out_t", [nqc, 512, QC], BF16,
                           kind="ExternalOutput").ap()
    # attention outputs are exchanged between pair cores by AllGather of y
    # (copy-only — much cheaper on the CC cores than ReduceScatter of proj
    # partials); each core then runs the full-1024-contraction projection
    # for its own 512 output features.  ag block 0 = heads 0-7 (the g=0
    # core's y), block 1 = heads 8-15 — identical layout on both cores.
    # the exchange is split: AG-A carries head pairs 0-2 (triggered ~75%
    # through the chunk), AG-B carries only pair 3 (at chunk end) — so the
    # tail only ever waits on a 128-row AllGather
    yx_d = [nc.dram_tensor(f"yx{q}", [512, QC], BF16).ap() for q in range(nqc)]
    aga_d = [nc.dram_tensor(f"aga{q}", [768, QC], BF16).ap()
             for q in range(nqc)]
    agb_d = [nc.dram_tensor(f"agb{q}", [256, QC], BF16).ap()
             for q in range(nqc)]
    zdr = nc.dram_tensor("zdr", [8, QC], FP32).ap()

    with tile.TileContext(nc) as tc, ExitStack() as ctx:
        resid = ctx.enter_context(tc.tile_pool(name="resid", bufs=1))
        xtp = ctx.enter_context(tc.tile_pool(name="xtp", bufs=2))
        qp = ctx.enter_context(tc.tile_pool(name="qp", bufs=2))
        yp = ctx.enter_context(tc.tile_pool(name="yp", bufs=2))
        ep = ctx.enter_context(tc.tile_pool(name="ep", bufs=2))
        rp = ctx.enter_context(tc.tile_pool(name="rp", bufs=4))
        op = ctx.enter_context(tc.tile_pool(name="op", bufs=2))
        agp = ctx.enter_context(tc.tile_pool(name="agp", bufs=2))
        ps_acc = ctx.enter_context(tc.tile_pool(name="ps_acc", bufs=1, space="PSUM"))
        ps_s = ctx.enter_context(tc.tile_pool(name="ps_s", bufs=2, space="PSUM"))
        ps_y = ctx.enter_context(tc.tile_pool(name="ps_y", bufs=3, space="PSUM"))

        # ---- startup DMAs: per-contraction-chunk tiles (fine-grained deps
        # so the first matmul starts after one slice pair, not the full set)
        # spread over three issue queues (sync: xt/bias, gpsimd: wv,
        # scalar: wqk) so the rings run in parallel
        bv_bc = resid.tile([128, 8, HD], FP32, name="bv_bc")
        xt0 = [xtp.tile([128, QC], BF16, name=f"xt{cch}", tag=f"xt{cch}")
               for cch in range(CCH)]
        wv_t = [resid.tile([128, 512], BF16, name=f"wv{cch}")
                for cch in range(CCH)]
        wqk_t = [resid.tile([128, 8, 128], BF16, name=f"wqk{cch}")
                 for cch in range(CCH)]
        for cch in range(CCH):
            nc.sync.dma_start(out=xt0[cch],
                              in_=xt_d[cch * 128:(cch + 1) * 128, 0:QC])
            nc.sync.dma_start(out=wv_t[cch],
                              in_=wv_d[cch * 128:(cch + 1) * 128, :])
            nc.scalar.dma_start(
                out=wqk_t[cch],
                in_=wqk_d[cch * 128:(cch + 1) * 128, :].rearrange(
                    "p (f n) -> p f n", f=8))
        nc.sync.dma_start(out=bv_bc, in_=bv_d)
        bqk_sb = resid.tile([128, 8], FP32, name="bqk_sb")
        nc.sync.dma_start(out=bqk_sb, in_=bqk_d.rearrange("(f p) -> p f", p=128))

        mask = resid.tile([128, 2, 128], BF16, name="mask")
        nc.sync.dma_start(out=mask, in_=mask_d)
        wp_sb = resid.tile([128, CCH, 4, 128], BF16, name="wp_sb")
        for cch in range(CCH):
            nc.sync.dma_start(
                out=wp_sb[:, cch],
                in_=wp_d[cch * 128:(cch + 1) * 128, :].rearrange(
                    "p (f n) -> p f n", f=4))
        bp_sb = resid.tile([128, 4], FP32, name="bp_sb")
        nc.sync.dma_start(out=bp_sb, in_=bp_d.rearrange("(f p) -> p f", p=128))

        ksb = [resid.tile([128, T], BF16, name=f"ksb{p}") for p in range(NP)]
        vsb = [resid.tile([128, 8, HD + 1], BF16, name=f"vsb{tb}")
               for tb in range(T // 128)]
        for tb in range(T // 128):
            nc.gpsimd.memset(vsb[tb][:, :, HD:HD + 1], 1.0)

        # ---------- emission helpers ----------
        def load_xt(qc):
            xt_sb = [xtp.tile([128, QC], BF16, name=f"xt{cch}",
                              tag=f"xt{cch}") for cch in range(CCH)]
            for cch in range(CCH):
                nc.sync.dma_start(
                    out=xt_sb[cch],
                    in_=xt_d[cch * 128:(cch + 1) * 128, qc * QC:(qc + 1) * QC])
            return xt_sb

        def acc_tile(alt):
            # ping-pong accumulator: ps_acc, or a borrowed half of a scores
            # bank (only safe where the scores pipeline isn't running hot)
            if alt % 2 == 0:
                return ps_acc.tile([128, QC], FP32, name="pacc", tag="pv")
            return ps_s.tile([128, 2 * QC], FP32, name="sps", tag="sps")[:, 0:QC]

        def emit_v(xt_sb, qc, j, alt=0):
            tb = qc * (QC // 128) + j
            pv = acc_tile(alt)
            for cch in range(CCH):
                nc.tensor.matmul(
                    out=pv, lhsT=xt_sb[cch][:, j * 128:(j + 1) * 128],
                    rhs=wv_t[cch], start=(cch == 0), stop=(cch == CCH - 1))
            nc.vector.tensor_add(
                vsb[tb][:, :, 0:HD],
                pv.rearrange("p (l d) -> p l d", l=8), bv_bc)

        def emit_qk(xt_sb, q_sb, qc, f, alt=0):
            pqk = acc_tile(alt)
            for cch in range(CCH):
                nc.tensor.matmul(
                    out=pqk, lhsT=wqk_t[cch][:, f], rhs=xt_sb[cch],
                    start=(cch == 0), stop=(cch == CCH - 1))
            p, isk = f // 2, f % 2
            dst = (ksb[p][:, qc * QC:(qc + 1) * QC] if isk else q_sb[p])
            nc.vector.tensor_scalar_add(dst, pqk, bqk_sb[:, f:f + 1])

        def emit_ag(qc, half):
            src = yx_d[qc][0:384, :] if half == 0 else yx_d[qc][384:512, :]
            dst = aga_d[qc] if half == 0 else agb_d[qc]
            nc.gpsimd.collective_compute(
                "AllGather", mybir.AluOpType.bypass, replica_groups=GROUPS,
                ins=[src], outs=[dst])

        # contraction slot order is rank-major = natural head order
        def ag_slice(qc, cch):
            if cch < 3:
                return aga_d[qc][cch * 128:(cch + 1) * 128, :]
            if cch == 3:
                return agb_d[qc][0:128, :]
            if cch < 7:
                return aga_d[qc][(cch - 1) * 128:cch * 128, :]
            return agb_d[qc][128:256, :]

        def load_ag(qc, cchs=range(CCH), scalar_ok=False):
            # separate tiles so each projection matmul only waits for its
            # own contraction slice.  Queue: gpsimd (idle except AG
            # triggers); in the epilogue the scalar queue is idle too.
            tiles = {}
            for cch in cchs:
                t = agp.tile([128, QC], BF16, name=f"ag{cch}", tag=f"ag{cch}")
                eng = nc.scalar if scalar_ok else nc.gpsimd
                eng.dma_start(out=t, in_=ag_slice(qc, cch))
                tiles[cch] = t
            return tiles

        def proj_pass(pp, ag_sb, oc, cchs, start, stop):
            for k, cch in enumerate(cchs):
                nc.tensor.matmul(out=pp, lhsT=wp_sb[:, cch, oc],
                                 rhs=ag_sb[cch],
                                 start=(start and k == 0),
                                 stop=(stop and k == len(cchs) - 1))

        def proj_evac(pp, qc, oc):
            po = op.tile([128, QC], BF16, name="po")
            nc.vector.tensor_scalar_add(po, pp, bp_sb[:, oc:oc + 1])
            nc.sync.dma_start(out=out_d[qc, oc * 128:(oc + 1) * 128, :],
                              in_=po)

        def emit_proj(ag_sb, qc, oc, alt=0):
            pp = acc_tile(alt)
            proj_pass(pp, ag_sb, oc, range(CCH), True, True)
            proj_evac(pp, qc, oc)

        def new_q():
            return [qp.tile([128, QC], BF16, name=f"qsb{p}", tag=f"qsb{p}")
                    for p in range(NP)]

        # ---------- prologue: chunk 0 qkv projection (ping-pong banks) ----
        xt_cur = xt0
        q_cur = new_q()
        alt = 0
        for j in range(QC // 128):
            emit_v(xt_cur, 0, j, alt); alt += 1
        for f in range(8):
            emit_qk(xt_cur, q_cur, 0, f, alt); alt += 1

        y_prev = None
        gstep = [0]
        dq = []     # deferred vector ops: (ready_gstep, fn) — issued a few
                    # steps after their z-broadcast DMA so they never sit
                    # waiting at the head of the vector queue

        def pop_dq():
            while dq and dq[0][0] <= gstep[0]:
                dq.pop(0)[1]()

        def flush_dq():
            while dq:
                dq.pop(0)[1]()

        for qc in range(nqc):
            # background PE units interleaved into this chunk's attention:
            # the previous chunk's AllGather-result load first (its waits sit
            # on the idle gpsimd queue), then next chunk's qkv, then the
            # previous chunk's projection (by which time the AG has landed)
            bg = []
            if qc > 0:
                ag_sb = load_ag(qc - 1)
            if qc + 1 < nqc:
                xt_nxt = load_xt(qc + 1)
                q_nxt = new_q()
                for j in range(QC // 128):
                    bg.append((emit_v, (xt_nxt, qc + 1, j)))
                for f in range(8):
                    bg.append((emit_qk, (xt_nxt, q_nxt, qc + 1, f)))
            else:
                xt_nxt, q_nxt = None, None
            if qc > 0:
                for oc in range(4):
                    bg.append((emit_proj, (ag_sb, qc - 1, oc)))
            bg_total = len(bg)

            njb = 4 * (qc + 1)
            steps = NP * njb
            # drain bg by ~7/8 of the chunk so end-of-chunk vector work
            # (normalize + AllGather feed) isn't stuck behind bg evacuations
            pace = max(1, (7 * steps) // 8)
            y_cur = [yp.tile([128, QC], BF16, name=f"y{p}", tag=f"y{p}")
                     for p in range(NP)]
            step = 0
            for p in range(NP):
                yps = [ps_y.tile([HD + 1, QC], FP32, name=f"yps{e}", tag="yps")
                       for e in (0, 1)]
                for jb in range(njb):
                    r = jb - 4 * qc
                    lo = max(0, 128 * r)      # first valid query column
                    sps = ps_s.tile([128, 2 * QC], FP32, name="sps", tag="sps")
                    for e in (0, 1):
                        nc.tensor.matmul(
                            out=sps[:, e * QC + lo:(e + 1) * QC],
                            lhsT=ksb[p][e * HD:(e + 1) * HD,
                                        jb * KB:(jb + 1) * KB],
                            rhs=q_cur[p][e * HD:(e + 1) * HD, lo:QC],
                            start=True, stop=True)
                    esb = ep.tile([128, 2, QC], BF16, name="esb")
                    nc.scalar.activation(
                        out=esb[:, :, lo:QC],
                        in_=sps.rearrange("p (e n) -> p e n", e=2)[:, :, lo:QC],
                        func=EXP, scale=0.125)
                    if r >= 0:
                        nc.vector.tensor_mul(
                            esb[:, :, lo:lo + 128], esb[:, :, lo:lo + 128],
                            mask)
                    for e in (0, 1):
                        nc.tensor.matmul(
                            out=yps[e][:, lo:QC], lhsT=vsb[jb][:, 2 * p + e, :],
                            rhs=esb[:, e, lo:QC],
                            start=(jb == 0), stop=(jb == njb - 1))
                    # keep the in-order PE stream dense: spread background
                    # units evenly across the attention steps
                    step += 1
                    gstep[0] += 1
                    pop_dq()
                    while bg and len(bg) > bg_total * max(0, pace - step) // pace:
                        fn, args = bg.pop(0)
                        fn(*args)
                # normalize: evacuate yps to SBUF right away (frees the PSUM
                # slots before the next p's matmuls need them), kick off the
                # Z DRAM-bounce broadcast, and defer reciprocal+multiply
                for e in (0, 1):
                    ysb = rp.tile([HD + 1, QC], FP32, name="ysb", tag="ysb")
                    nc.vector.tensor_copy(ysb, yps[e])
                    slot = zdr[2 * p + e]
                    nc.sync.dma_start(out=slot, in_=ysb[HD:HD + 1, :])
                    rzb = rp.tile([HD, QC], FP32, name="rzb", tag="rzb")
                    nc.sync.dma_start(out=rzb,
                                      in_=slot.partition_broadcast(HD))

                    def mk(ysb=ysb, rzb=rzb, p=p, e=e, y_cur=y_cur, qc=qc):
                        def go():
                            nc.vector.reciprocal_approx_fast(rzb, rzb)
                            nc.vector.tensor_mul(
                                y_cur[p][e * HD:(e + 1) * HD, :],
                                ysb[0:HD, :], rzb)
                            nc.sync.dma_start(
                                out=yx_d[qc][p * 128 + e * HD:
                                             p * 128 + (e + 1) * HD, :],
                                in_=y_cur[p][e * HD:(e + 1) * HD, :])
                            if p == 2 and e == 1:
                                emit_ag(qc, 0)
                        return go
                    dq.append((gstep[0] + 3, mk()))
            for fn, args in bg:
                fn(*args)
            flush_dq()
            emit_ag(qc, 1)
            y_prev = y_cur
            xt_cur, q_cur = xt_nxt, q_nxt

        # epilogue, two passes: the AG-A slices (own/peer pairs 0-2) project
        # while pair 3's normalize chain and its small AG-B run; pass 2 adds
        # the two pair-3 slices.  4 simultaneously-open PSUM groups (the
        # attention banks are free by now).
        A_CCHS, B_CCHS = [0, 1, 2, 4, 5, 6], [3, 7]
        lq = nqc - 1
        ag_sb = load_ag(lq, A_CCHS, scalar_ok=True)
        spsA = ps_s.tile([128, 2 * QC], FP32, name="sps", tag="sps")
        spsB = ps_s.tile([128, 2 * QC], FP32, name="sps", tag="sps")
        pps = [ps_acc.tile([128, QC], FP32, name="pacc", tag="pv"),
               spsA[:, 0:QC], spsA[:, QC:2 * QC], spsB[:, 0:QC]]
        for oc in range(4):
            proj_pass(pps[oc], ag_sb, oc, A_CCHS, True, False)
        ag_sb.update(load_ag(lq, B_CCHS, scalar_ok=True))
        for oc in range(4):
            proj_pass(pps[oc], ag_sb, oc, B_CCHS, False, True)
            proj_evac(pps[oc], lq, oc)


    nc.compile()
    return nc


def shard_inputs(x, W_attn, b_attn, W_proj, b_proj):
    in_maps = []
    u = np.arange(128)[None, :]
    p_ = np.arange(128)[:, None]
    tri = (p_ <= u).astype(BF)                     # [128, 128] lower-tri
    mask_np = np.ascontiguousarray(
        np.broadcast_to(tri[:, None, :], (128, 2, 128)))
    wp_perm = np.concatenate([np.arange(0, 256), np.arange(512, 768),
                              np.arange(256, 512), np.arange(768, 1024)])
    for c in range(NCORES):
        b, g = c // 2, c % 2
        xt = np.ascontiguousarray(x[b].T.astype(BF))
        # w_qk columns: feat chunk f = 2p+isK holds q (isK=0) or k (isK=1)
        # features of heads (8g+2p, 8g+2p+1)
        qk_idx = []
        for f in range(8):
            p, isk = f // 2, f % 2
            for e in (0, 1):
                h = 8 * g + 2 * p + e
                base = isk * C + h * HD
                qk_idx.append(np.arange(base, base + HD))
        qk_idx = np.concatenate(qk_idx)
        v_idx = np.concatenate(
            [np.arange(2 * C + (8 * g + l) * HD, 2 * C + (8 * g + l) * HD + HD)
             for l in range(8)])
        in_maps.append({
            "mask": mask_np,
            "xt": xt,
            "wqk": np.ascontiguousarray(W_attn[:, qk_idx].astype(BF)),
            "bqk": np.ascontiguousarray(b_attn[qk_idx].astype(np.float32)),
            "wv": np.ascontiguousarray(W_attn[:, v_idx].astype(BF)),
            "bv": np.ascontiguousarray(np.broadcast_to(
                b_attn[v_idx].astype(np.float32).reshape(8, HD),
                (128, 8, HD))),
            # full contraction rows (ag slots are rank-ordered = natural
            # head order on both cores); this core's output columns
            "wp": np.ascontiguousarray(
                W_proj[:, g * 512:(g + 1) * 512].astype(BF)),
            "bp": np.ascontiguousarray(
                b_proj[g * 512:(g + 1) * 512].astype(np.float32)),
        })
    return in_maps


def assemble_output(results, T):
    out = np.empty((B, T, C), np.float32)
    nqc = T // QC
    for b in range(B):
        for g in range(2):
            ot = np.asarray(results[2 * b + g]["out_t"], dtype=np.float32)
            for qc in range(nqc):
                out[b, qc * QC:(qc + 1) * QC, g * 512:(g + 1) * 512] = \
                    ot[qc].T
    return out


_PROG = {}


def _get_program(T, mode="full"):
    key = (T, mode)
    if key not in _PROG:
        _PROG[key] = build_program(T, mode)
    return _PROG[key]


def run_sharded(inputs, trace=False, mode="full"):
    """Returns (output [B,T,C], BassKernelResults)."""
    x = np.asarray(inputs["x"])
    T = x.shape[1]
    nc = _get_program(T, mode)
    in_maps = shard_inputs(x, np.asarray(inputs["W_attn"]),
                           np.asarray(inputs["b_attn"]),
                           np.asarray(inputs["W_proj"]),
                           np.asarray(inputs["b_proj"]))
    res = run_bass_kernel_spmd(nc, in_maps, list(range(NCORES)), trace=trace)
    return assemble_output(res.results, T), res


def kernel(**inputs):
    out, _ = run_sharded(inputs)
    return out
